# revision 15
# baseline (speedup 1.0000x reference)
"""Trainium2 Bass kernel for nn_BoundaryLoss (3D boundary/dice loss).

Math: for pred/target volumes [2,1,192,192,192] f32,
  b(x) = sqrt(gx^2+gy^2+gz^2+1e-5) with central differences (zero pad),
  loss = 1 - (2*sum(pb*tb)+s)/(sum(pb)+sum(tb)+s).

Sharding: 8 cores = 2 batches x 4 depth-quarters (48 slices each, 1-slice
halo).  Each core computes 3 partial sums; host combines.

Per-core layout: a tensor shard is [H=192 rows, 50 slices x 196 cols] fp16
(W padded 192->196 with zeros at cols {0,1,194,195}; data col j = w+2).
H is split into chunk A (partitions 0..127, valid h 0..126) and chunk B
(rows 120..191 on 72 partitions, valid h 127..191).  With (d,w) flattened
on the free axis:
  gx (depth diff)  = flat shift by +-196  -> fused sq-diff on DVE
  gz (width diff)  = flat shift by +-1    -> fused sq-diff on DVE
  gy (height diff) = partition shift      -> PE matmul with +-1 shift matrix
Then V = gx2+gz2+eps+gy2 (DVE add + scalar_tensor_tensor), pb = ACT
Sqrt(V) with per-partition accum (sum pb), and sum(pb*tb) via DVE
tensor_tensor_reduce.  All accumulator slots are f32; host sums in f64.
"""

import sys

sys.path.insert(0, "/opt/trn_rl_repo")

import numpy as np

# ---------------- problem constants (hardcoded per contract) ----------------
BATCH = 2
DVOL = 192           # full depth
H = 192
W = 192
NCORES = 8
NQ = 4               # depth quarters per batch
DL = DVOL // NQ      # 48 local slices per core
S = DL + 2           # 50 slices incl halo
WP = W + 4           # 196 padded row
FREE = S * WP        # 9800
OUTC = DL * WP       # 9408 output cols per chunk
SBC = 1568           # sub-block cols (8 slices x 196)
NSB = OUTC // SBC    # 6
SLICES_PER_SB = SBC // WP  # 8
EPS = 1e-5
B0 = 120             # chunk B first H row
PA, PB_ = 128, 72    # partitions per chunk
# valid partition ranges [lo, hi) for accumulation
VA = (0, 127)        # chunk A covers h 0..126
VB = (7, 72)         # chunk B covers h 127..191

_NC_CACHE = {}

# this container's walrus rejects instructions carrying more than a couple
# of semaphore waits ("Too many sync wait commands" on the Tile tail drain).
# Split excess waits onto same-engine Drain instructions inserted just
# before the offender, at the serialized-BIR level (single choke point for
# both the PJRT/axon path and compile_bass_kernel).
_WAIT_CAP = 1


def _split_multiwait_json(bs: bytes) -> bytes:
    import json

    m = json.loads(bs)
    changed = False
    for fn in m.get("functions", []):
        for blk in fn.get("blocks", []):
            insts = blk.get("instructions")
            if not insts:
                continue
            out = []
            for ins in insts:
                si = ins.get("sync_info") or {}
                ow = si.get("on_wait") or []
                if len(ow) > _WAIT_CAP:
                    chunks = [
                        ow[i : i + _WAIT_CAP] for i in range(0, len(ow), _WAIT_CAP)
                    ]
                    for ci, ch in enumerate(chunks[:-1]):
                        out.append(
                            {
                                "debug": ins.get("debug", 0),
                                "engine": ins["engine"],
                                "ins": [],
                                "outs": [],
                                "is_reset_sema": False,
                                "name": f"{ins['name']}__w{ci}",
                                "opcode": "Drain",
                                "sync_info": {"on_update": [], "on_wait": ch},
                            }
                        )
                    si["on_wait"] = chunks[-1]
                    ins["sync_info"] = si
                    changed = True
                out.append(ins)
            blk["instructions"] = out
    if not changed:
        return bs
    return json.dumps(m).encode()


def _install_json_patch():
    import concourse.bass as bass

    if getattr(bass.Bass, "_bl_json_patched", False):
        return
    orig = bass.Bass.to_json_bytes

    def to_json_bytes(self, *a, **k):
        return _split_multiwait_json(orig(self, *a, **k))

    bass.Bass.to_json_bytes = to_json_bytes
    bass.Bass._bl_json_patched = True


# ---------------- custom DVE op: out = (in0 - in1)^2 ----------------
def _register_sqdiff():
    import concourse.dve_ops as dve_ops
    from concourse.dve_spec import Spec, Src0, Src1, lower, sq
    from concourse.dve_uop import DveOpSpec

    name = "SQDIFF_BL"
    for op in dve_ops.OPS:
        if op.name == name:
            return op
    spec = Spec(
        body=sq(Src0 - Src1),
        reference=lambda in0, in1, s0, s1, imm2: (
            in0.astype(np.float32) - in1.astype(np.float32)
        )
        ** 2,
    )
    shas = {}
    for ver in ("v3", "v4"):
        s = DveOpSpec(name=name, opcode=1, uops=lower(spec, ver=ver), rd1_en=True)
        shas[ver] = s.sha(ver)
    op = dve_ops.DveOp(name, spec, subdim=False, uops_sha=shas)
    row = max(dve_ops._SUB_OPCODE_FOR_NAME.values()) + 1
    assert row < 0x20
    dve_ops.OPS.append(op)
    dve_ops.CUSTOM_DVE_SPECS[name] = spec
    dve_ops._SUB_OPCODE_FOR_NAME[name] = row
    return op


# ---------------- device program ----------------
def build_nc():
    from contextlib import ExitStack

    import concourse.bass as bass
    import concourse.mybir as mybir
    from concourse import tile

    _install_json_patch()

    f16 = mybir.dt.float16
    f32 = mybir.dt.float32
    ADD = mybir.AluOpType.add
    MULT = mybir.AluOpType.mult
    SQUARE = mybir.ActivationFunctionType.Square
    SQRT = mybir.ActivationFunctionType.Sqrt
    AXX = mybir.AxisListType.X

    nc = bass.Bass("TRN2", target_bir_lowering=False, debug=False)

    xp = nc.dram_tensor("xp", [H, FREE], f16, kind="ExternalInput")
    xt = nc.dram_tensor("xt", [H, FREE], f16, kind="ExternalInput")
    da = nc.dram_tensor("da", [PA, PA], f16, kind="ExternalInput")
    db = nc.dram_tensor("db", [PB_, PB_], f16, kind="ExternalInput")
    out = nc.dram_tensor("o", [128, 8], f32, kind="ExternalOutput")

    # matmul windows within one 1568-col sub-block (each inside one PSUM bank)
    MMW = [(0, 512), (512, 512), (1024, 512), (1536, 32)]

    with tile.TileContext(nc) as tc, ExitStack() as ctx:
        const = ctx.enter_context(tc.tile_pool(name="const", bufs=1))
        xpool = ctx.enter_context(tc.tile_pool(name="x", bufs=1))
        work = ctx.enter_context(tc.tile_pool(name="work", bufs=3))
        pbp = ctx.enter_context(tc.tile_pool(name="pb", bufs=2))
        accp = ctx.enter_context(tc.tile_pool(name="acc", bufs=1))
        psum = ctx.enter_context(tc.tile_pool(name="psum", bufs=2, space="PSUM"))

        da_t = const.tile([PA, PA], f16, tag="da")
        nc.sync.dma_start(da_t[:], da[:, :])
        db_t = const.tile([PB_, PB_], f16, tag="db")
        nc.sync.dma_start(db_t[:], db[:, :])

        X = {}
        for tname, dram in (("p", xp), ("t", xt)):
            for ch, pc, r0 in (("A", PA, 0), ("B", PB_, B0)):
                t_ = xpool.tile([pc, FREE], f16, tag=f"x{tname}{ch}")
                # split the load for earlier pipeline start
                half = FREE // 2
                nc.sync.dma_start(t_[:, 0:half], dram[r0 : r0 + pc, 0:half])
                nc.sync.dma_start(t_[:, half:FREE], dram[r0 : r0 + pc, half:FREE])
                X[tname, ch] = t_

        # accumulator slot tiles: per (quantity, chunk), one f32 col per sub-block
        SA = {}
        for q in ("sp", "st", "pt"):
            for ch in ("A", "B"):
                SA[q, ch] = accp.tile(
                    [128, NSB], f32, tag=f"sa_{q}_{ch}", name=f"sa_{q}_{ch}"
                )

        for ch, pc, dmat, (vlo, vhi) in (
            ("A", PA, da_t, VA),
            ("B", PB_, db_t, VB),
        ):
            for sb in range(NSB):
                c0 = sb * SBC
                PBt = {}
                for tname in ("p", "t"):
                    x = X[tname, ch]
                    # gx^2: depth central diff, flat shift +-196
                    gx = work.tile([pc, SBC], f16, tag="gx")
                    nc.vector.tensor_sub(
                        gx[:],
                        x[:, c0 + 392 : c0 + 392 + SBC],
                        x[:, c0 : c0 + SBC],
                    )
                    gx2 = work.tile([pc, SBC], f16, tag="gx2")
                    nc.vector.tensor_mul(gx2[:], gx[:], gx[:])
                    # gz^2: width central diff, flat shift +-1 (center +196)
                    gz = work.tile([pc, SBC], f16, tag="gz")
                    nc.vector.tensor_sub(
                        gz[:],
                        x[:, c0 + 197 : c0 + 197 + SBC],
                        x[:, c0 + 195 : c0 + 195 + SBC],
                    )
                    gz2 = work.tile([pc, SBC], f16, tag="gz2")
                    nc.vector.tensor_mul(gz2[:], gz[:], gz[:])
                    # gy via PE shift-matmul into PSUM, then ACT square
                    ps = psum.tile([pc, SBC], f32, tag="ps")
                    for w0, wn in MMW:
                        nc.tensor.matmul(
                            ps[:, w0 : w0 + wn],
                            dmat[:],
                            x[:, 196 + c0 + w0 : 196 + c0 + w0 + wn],
                            start=True,
                            stop=True,
                        )
                    q_ = work.tile([pc, SBC], f16, tag="q")
                    nc.scalar.activation(q_[:], ps[:], SQUARE)
                    # v0 = gx2 + gz2 ; v1 = (v0 + eps) + gy2
                    v0 = work.tile([pc, SBC], f16, tag="v0")
                    nc.vector.tensor_add(v0[:], gx2[:], gz2[:])
                    v1 = work.tile([pc, SBC], f16, tag="v1")
                    nc.vector.scalar_tensor_tensor(
                        v1[:], v0[:], EPS, q_[:], op0=ADD, op1=ADD
                    )
                    # pb = sqrt(v1) on data cols only, accum = per-partition sum
                    pb = pbp.tile([pc, SLICES_PER_SB * W], f16, tag=f"pb{tname}")
                    v3 = v1[:].rearrange("p (s w) -> p s w", s=SLICES_PER_SB)
                    pb3 = pb[:].rearrange("p (s w) -> p s w", s=SLICES_PER_SB)
                    qn = "sp" if tname == "p" else "st"
                    nc.scalar.activation(
                        pb3[:, :, :],
                        v3[:, :, 2 : 2 + W],
                        SQRT,
                        accum_out=SA[qn, ch][0:pc, sb : sb + 1],
                    )
                    PBt[tname] = pb
                # sum(pb*tb) for this sub-block: (pb*1.0)*tb with fused accum
                prod = work.tile([pc, SLICES_PER_SB * W], f16, tag="prod")
                nc.vector.scalar_tensor_tensor(
                    prod[:, :],
                    PBt["p"][:, :],
                    1.0,
                    PBt["t"][:, :],
                    op0=MULT,
                    op1=MULT,
                    accum_out=SA["pt", ch][0:pc, sb : sb + 1],
                )

        # reduce slot columns and write partials to DRAM
        colmap = [
            ("sp", "A"), ("sp", "B"),
            ("st", "A"), ("st", "B"),
            ("pt", "A"), ("pt", "B"),
        ]
        for col, (q, ch) in enumerate(colmap):
            vlo, vhi = VA if ch == "A" else VB
            pc = PA if ch == "A" else PB_
            r = accp.tile([128, 1], f32, tag=f"red{col}")
            nc.vector.tensor_reduce(r[0:pc, :], SA[q, ch][0:pc, :], AXX, ADD)
            nc.sync.dma_start(out[vlo:vhi, col : col + 1], r[vlo:vhi, :])

    return nc


def get_nc():
    if "nc" not in _NC_CACHE:
        _NC_CACHE["nc"] = build_nc()
    return _NC_CACHE["nc"]


# ---------------- host-side sharding ----------------
def _dmat(k):
    d = np.zeros((k, k), np.float16)
    for m in range(k):
        if m + 1 < k:
            d[m + 1, m] = 1.0
        if m - 1 >= 0:
            d[m - 1, m] = -1.0
    return d


DA_NP = _dmat(PA)
DB_NP = _dmat(PB_)


def _shard(vol, q):
    """vol [192,192,192] f32 -> [H, FREE] fp16 padded shard for quarter q."""
    sh = np.zeros((S, H, WP), np.float16)
    d0 = DL * q - 1
    lo, hi = max(d0, 0), min(d0 + S, DVOL)
    sh[lo - d0 : hi - d0, :, 2 : 2 + W] = vol[lo:hi].astype(np.float16)
    # -> [H, S, WP] -> [H, FREE]
    return np.ascontiguousarray(sh.transpose(1, 0, 2)).reshape(H, FREE)


def make_in_maps(pred, target):
    pred = np.asarray(pred, dtype=np.float32).reshape(BATCH, DVOL, H, W)
    target = np.asarray(target, dtype=np.float32).reshape(BATCH, DVOL, H, W)
    maps = []
    for c in range(NCORES):
        b, q = divmod(c, NQ)
        maps.append(
            {
                "xp": _shard(pred[b], q),
                "xt": _shard(target[b], q),
                "da": DA_NP,
                "db": DB_NP,
            }
        )
    return maps


def combine(results):
    sp = st = pt = 0.0
    a0, a1 = VA
    b0, b1 = VB
    for r in results:
        o = np.asarray(r["o"], dtype=np.float64)
        sp += o[a0:a1, 0].sum() + o[b0:b1, 1].sum()
        st += o[a0:a1, 2].sum() + o[b0:b1, 3].sum()
        pt += o[a0:a1, 4].sum() + o[b0:b1, 5].sum()
    dice = (2.0 * pt + EPS) / (sp + st + EPS)
    return np.float32(1.0 - dice)


def run_on_device(in_maps, **kwargs):
    from concourse.bass_utils import run_bass_kernel_spmd

    nc = get_nc()
    return run_bass_kernel_spmd(nc, in_maps, core_ids=list(range(NCORES)), **kwargs)


def kernel(pred, target):
    in_maps = make_in_maps(pred, target)
    res = run_on_device(in_maps)
    return combine(res.results)


if __name__ == "__main__":
    rng = np.random.default_rng(0)
    p = rng.random((2, 1, 192, 192, 192), np.float32)
    t = rng.random((2, 1, 192, 192, 192), np.float32)
    print(kernel(p, t))


# revision 17
# speedup vs baseline: 1.1466x; 1.1466x over previous
"""Trainium2 Bass kernel for nn_BoundaryLoss (3D boundary/dice loss).

Math: for pred/target volumes [2,1,192,192,192] f32,
  b(x) = sqrt(gx^2+gy^2+gz^2+1e-5) with central differences (zero pad),
  loss = 1 - (2*sum(pb*tb)+s)/(sum(pb)+sum(tb)+s).

Sharding: 8 cores = 2 batches x 4 depth-quarters (48 slices each, 1-slice
halo).  Each core computes 3 partial sums; host combines.

Per-core layout: a tensor shard is [H=192 rows, 50 slices x 196 cols] fp16
(W padded 192->196 with zeros at cols {0,1,194,195}; data col j = w+2).
H is split into chunk A (partitions 0..127, valid h 0..126) and chunk B
(rows 120..191 on 72 partitions, valid h 127..191).  With (d,w) flattened
on the free axis:
  gx (depth diff)  = flat shift by +-196  -> fused sq-diff on DVE
  gz (width diff)  = flat shift by +-1    -> fused sq-diff on DVE
  gy (height diff) = partition shift      -> PE matmul with +-1 shift matrix
Then V = gx2+gz2+eps+gy2 (DVE add + scalar_tensor_tensor), pb = ACT
Sqrt(V) with per-partition accum (sum pb), and sum(pb*tb) via DVE
tensor_tensor_reduce.  All accumulator slots are f32; host sums in f64.
"""

import sys

sys.path.insert(0, "/opt/trn_rl_repo")

import numpy as np

# ---------------- problem constants (hardcoded per contract) ----------------
BATCH = 2
DVOL = 192           # full depth
H = 192
W = 192
NCORES = 8
NQ = 4               # depth quarters per batch
DL = DVOL // NQ      # 48 local slices per core
S = DL + 2           # 50 slices incl halo
WP = W + 4           # 196 padded row
FREE = S * WP        # 9800
OUTC = DL * WP       # 9408 output cols per chunk
SBC = 1568           # sub-block cols (8 slices x 196)
NSB = OUTC // SBC    # 6
SLICES_PER_SB = SBC // WP  # 8
EPS = 1e-5
B0 = 120             # chunk B first H row
PA, PB_ = 128, 72    # partitions per chunk
# valid partition ranges [lo, hi) for accumulation
VA = (0, 127)        # chunk A covers h 0..126
VB = (7, 72)         # chunk B covers h 127..191

_NC_CACHE = {}

# this container's walrus rejects instructions carrying more than a couple
# of semaphore waits ("Too many sync wait commands" on the Tile tail drain).
# Split excess waits onto same-engine Drain instructions inserted just
# before the offender, at the serialized-BIR level (single choke point for
# both the PJRT/axon path and compile_bass_kernel).
_WAIT_CAP = 1


def _split_multiwait_json(bs: bytes) -> bytes:
    import json

    m = json.loads(bs)
    changed = False
    for fn in m.get("functions", []):
        for blk in fn.get("blocks", []):
            insts = blk.get("instructions")
            if not insts:
                continue
            out = []
            for ins in insts:
                si = ins.get("sync_info") or {}
                ow = si.get("on_wait") or []
                if len(ow) > _WAIT_CAP:
                    chunks = [
                        ow[i : i + _WAIT_CAP] for i in range(0, len(ow), _WAIT_CAP)
                    ]
                    for ci, ch in enumerate(chunks[:-1]):
                        out.append(
                            {
                                "debug": ins.get("debug", 0),
                                "engine": ins["engine"],
                                "ins": [],
                                "outs": [],
                                "is_reset_sema": False,
                                "name": f"{ins['name']}__w{ci}",
                                "opcode": "EventSemaphore",
                                "sync_info": {"on_update": [], "on_wait": ch},
                            }
                        )
                    si["on_wait"] = chunks[-1]
                    ins["sync_info"] = si
                    changed = True
                out.append(ins)
            blk["instructions"] = out
    if not changed:
        return bs
    return json.dumps(m).encode()


def _install_json_patch():
    import concourse.bass as bass

    if getattr(bass.Bass, "_bl_json_patched", False):
        return
    orig = bass.Bass.to_json_bytes

    def to_json_bytes(self, *a, **k):
        return _split_multiwait_json(orig(self, *a, **k))

    bass.Bass.to_json_bytes = to_json_bytes
    bass.Bass._bl_json_patched = True


# ---------------- custom DVE op: out = (in0 - in1)^2 ----------------
def _register_sqdiff():
    import concourse.dve_ops as dve_ops
    from concourse.dve_spec import Spec, Src0, Src1, lower, sq
    from concourse.dve_uop import DveOpSpec

    name = "SQDIFF_BL"
    for op in dve_ops.OPS:
        if op.name == name:
            return op
    spec = Spec(
        body=sq(Src0 - Src1),
        reference=lambda in0, in1, s0, s1, imm2: (
            in0.astype(np.float32) - in1.astype(np.float32)
        )
        ** 2,
    )
    shas = {}
    for ver in ("v3", "v4"):
        s = DveOpSpec(name=name, opcode=1, uops=lower(spec, ver=ver), rd1_en=True)
        shas[ver] = s.sha(ver)
    op = dve_ops.DveOp(name, spec, subdim=False, uops_sha=shas)
    row = max(dve_ops._SUB_OPCODE_FOR_NAME.values()) + 1
    assert row < 0x20
    dve_ops.OPS.append(op)
    dve_ops.CUSTOM_DVE_SPECS[name] = spec
    dve_ops._SUB_OPCODE_FOR_NAME[name] = row
    return op


# ---------------- device program ----------------
def build_nc():
    from contextlib import ExitStack

    import concourse.bass as bass
    import concourse.mybir as mybir
    from concourse import tile

    _install_json_patch()

    f16 = mybir.dt.float16
    f32 = mybir.dt.float32
    ADD = mybir.AluOpType.add
    MULT = mybir.AluOpType.mult
    SQUARE = mybir.ActivationFunctionType.Square
    SQRT = mybir.ActivationFunctionType.Sqrt
    AXX = mybir.AxisListType.X

    nc = bass.Bass("TRN2", target_bir_lowering=False, debug=False)

    xp = nc.dram_tensor("xp", [H, FREE], f16, kind="ExternalInput")
    xt = nc.dram_tensor("xt", [H, FREE], f16, kind="ExternalInput")
    da = nc.dram_tensor("da", [PA, PA], f16, kind="ExternalInput")
    db = nc.dram_tensor("db", [PB_, PB_], f16, kind="ExternalInput")
    out = nc.dram_tensor("o", [128, 8], f32, kind="ExternalOutput")

    # matmul windows within one 1568-col sub-block (each inside one PSUM bank)
    MMW = [(0, 512), (512, 512), (1024, 512), (1536, 32)]

    with tile.TileContext(nc) as tc, ExitStack() as ctx:
        const = ctx.enter_context(tc.tile_pool(name="const", bufs=1))
        xpool = ctx.enter_context(tc.tile_pool(name="x", bufs=1))
        work = ctx.enter_context(tc.tile_pool(name="work", bufs=3))
        pbp = ctx.enter_context(tc.tile_pool(name="pb", bufs=2))
        accp = ctx.enter_context(tc.tile_pool(name="acc", bufs=1))
        psum = ctx.enter_context(tc.tile_pool(name="psum", bufs=2, space="PSUM"))

        da_t = const.tile([PA, PA], f16, tag="da")
        nc.sync.dma_start(da_t[:], da[:, :])
        db_t = const.tile([PB_, PB_], f16, tag="db")
        nc.sync.dma_start(db_t[:], db[:, :])

        X = {}
        for tname, dram in (("p", xp), ("t", xt)):
            for ch, pc, r0 in (("A", PA, 0), ("B", PB_, B0)):
                t_ = xpool.tile([pc, FREE], f16, tag=f"x{tname}{ch}")
                # split the load for earlier pipeline start
                half = FREE // 2
                nc.sync.dma_start(t_[:, 0:half], dram[r0 : r0 + pc, 0:half])
                nc.sync.dma_start(t_[:, half:FREE], dram[r0 : r0 + pc, half:FREE])
                X[tname, ch] = t_

        # accumulator slot tiles: per (quantity, chunk), one f32 col per sub-block
        SA = {}
        for q in ("sp", "st", "pt"):
            for ch in ("A", "B"):
                SA[q, ch] = accp.tile(
                    [128, NSB], f32, tag=f"sa_{q}_{ch}", name=f"sa_{q}_{ch}"
                )

        for ch, pc, dmat, (vlo, vhi) in (
            ("A", PA, da_t, VA),
            ("B", PB_, db_t, VB),
        ):
            for sb in range(NSB):
                c0 = sb * SBC
                PBt = {}
                for tname in ("p", "t"):
                    x = X[tname, ch]
                    # gx^2: depth central diff, flat shift +-196
                    gx = work.tile([pc, SBC], f16, tag="gx")
                    nc.vector.tensor_sub(
                        gx[:],
                        x[:, c0 + 392 : c0 + 392 + SBC],
                        x[:, c0 : c0 + SBC],
                    )
                    gx2 = work.tile([pc, SBC], f16, tag="gx2")
                    nc.vector.tensor_mul(gx2[:], gx[:], gx[:])
                    # gz^2: width central diff, flat shift +-1 (center +196)
                    gz = work.tile([pc, SBC], f16, tag="gz")
                    nc.vector.tensor_sub(
                        gz[:],
                        x[:, c0 + 197 : c0 + 197 + SBC],
                        x[:, c0 + 195 : c0 + 195 + SBC],
                    )
                    gz2 = work.tile([pc, SBC], f16, tag="gz2")
                    nc.vector.tensor_mul(gz2[:], gz[:], gz[:])
                    # gy via PE shift-matmul into PSUM, then ACT square
                    ps = psum.tile([pc, SBC], f32, tag="ps")
                    for w0, wn in MMW:
                        nc.tensor.matmul(
                            ps[:, w0 : w0 + wn],
                            dmat[:],
                            x[:, 196 + c0 + w0 : 196 + c0 + w0 + wn],
                            start=True,
                            stop=True,
                        )
                    q_ = work.tile([pc, SBC], f16, tag="q")
                    nc.scalar.activation(q_[:], ps[:], SQUARE)
                    # v0 = gx2 + gz2 ; v1 = (v0 + eps) + gy2
                    v0 = work.tile([pc, SBC], f16, tag="v0")
                    nc.vector.tensor_add(v0[:], gx2[:], gz2[:])
                    v1 = work.tile([pc, SBC], f16, tag="v1")
                    nc.vector.scalar_tensor_tensor(
                        v1[:], v0[:], EPS, q_[:], op0=ADD, op1=ADD
                    )
                    # pb = sqrt(v1) on data cols only, accum = per-partition sum
                    pb = pbp.tile([pc, SLICES_PER_SB * W], f16, tag=f"pb{tname}")
                    v3 = v1[:].rearrange("p (s w) -> p s w", s=SLICES_PER_SB)
                    pb3 = pb[:].rearrange("p (s w) -> p s w", s=SLICES_PER_SB)
                    qn = "sp" if tname == "p" else "st"
                    nc.scalar.activation(
                        pb3[:, :, :],
                        v3[:, :, 2 : 2 + W],
                        SQRT,
                        accum_out=SA[qn, ch][0:pc, sb : sb + 1],
                    )
                    PBt[tname] = pb
                # sum(pb*tb) for this sub-block: (pb*1.0)*tb with fused accum
                prod = work.tile([pc, SLICES_PER_SB * W], f16, tag="prod")
                nc.vector.scalar_tensor_tensor(
                    prod[:, :],
                    PBt["p"][:, :],
                    1.0,
                    PBt["t"][:, :],
                    op0=MULT,
                    op1=MULT,
                    accum_out=SA["pt", ch][0:pc, sb : sb + 1],
                )

        # reduce slot columns and write partials to DRAM
        colmap = [
            ("sp", "A"), ("sp", "B"),
            ("st", "A"), ("st", "B"),
            ("pt", "A"), ("pt", "B"),
        ]
        for col, (q, ch) in enumerate(colmap):
            vlo, vhi = VA if ch == "A" else VB
            pc = PA if ch == "A" else PB_
            r = accp.tile([128, 1], f32, tag=f"red{col}")
            nc.vector.tensor_reduce(r[0:pc, :], SA[q, ch][0:pc, :], AXX, ADD)
            nc.sync.dma_start(out[vlo:vhi, col : col + 1], r[vlo:vhi, :])

    return nc


def get_nc():
    if "nc" not in _NC_CACHE:
        _NC_CACHE["nc"] = build_nc()
    return _NC_CACHE["nc"]


# ---------------- host-side sharding ----------------
def _dmat(k):
    d = np.zeros((k, k), np.float16)
    for m in range(k):
        if m + 1 < k:
            d[m + 1, m] = 1.0
        if m - 1 >= 0:
            d[m - 1, m] = -1.0
    return d


DA_NP = _dmat(PA)
DB_NP = _dmat(PB_)


def _shard(vol, q):
    """vol [192,192,192] f32 -> [H, FREE] fp16 padded shard for quarter q."""
    sh = np.zeros((S, H, WP), np.float16)
    d0 = DL * q - 1
    lo, hi = max(d0, 0), min(d0 + S, DVOL)
    sh[lo - d0 : hi - d0, :, 2 : 2 + W] = vol[lo:hi].astype(np.float16)
    # -> [H, S, WP] -> [H, FREE]
    return np.ascontiguousarray(sh.transpose(1, 0, 2)).reshape(H, FREE)


def make_in_maps(pred, target):
    pred = np.asarray(pred, dtype=np.float32).reshape(BATCH, DVOL, H, W)
    target = np.asarray(target, dtype=np.float32).reshape(BATCH, DVOL, H, W)
    maps = []
    for c in range(NCORES):
        b, q = divmod(c, NQ)
        maps.append(
            {
                "xp": _shard(pred[b], q),
                "xt": _shard(target[b], q),
                "da": DA_NP,
                "db": DB_NP,
            }
        )
    return maps


def combine(results):
    sp = st = pt = 0.0
    a0, a1 = VA
    b0, b1 = VB
    for r in results:
        o = np.asarray(r["o"], dtype=np.float64)
        sp += o[a0:a1, 0].sum() + o[b0:b1, 1].sum()
        st += o[a0:a1, 2].sum() + o[b0:b1, 3].sum()
        pt += o[a0:a1, 4].sum() + o[b0:b1, 5].sum()
    dice = (2.0 * pt + EPS) / (sp + st + EPS)
    return np.float32(1.0 - dice)


def run_on_device(in_maps, **kwargs):
    from concourse.bass_utils import run_bass_kernel_spmd

    nc = get_nc()
    return run_bass_kernel_spmd(nc, in_maps, core_ids=list(range(NCORES)), **kwargs)


def kernel(pred, target):
    in_maps = make_in_maps(pred, target)
    res = run_on_device(in_maps)
    return combine(res.results)


if __name__ == "__main__":
    rng = np.random.default_rng(0)
    p = rng.random((2, 1, 192, 192, 192), np.float32)
    t = rng.random((2, 1, 192, 192, 192), np.float32)
    print(kernel(p, t))
